# revision 8
# baseline (speedup 1.0000x reference)
"""CenterLoss kernel for Trainium2 (8 NeuronCores, data-parallel over W).

loss = sum_{n,c,w} act[n,c,w] * dist[n,c,w],  clipped at 1e-6, where
  dist[n,c,w] = ||x[n,:,w] - ctr[:,c]||^2 = x2[n,w] - 2*xc[n,c,w] + c2[c]

Sharding: each core takes a W/8 = 2048-column slice (all 16 n, all 80 c).
Both x and act are pre-cast to bf16 on the host, so every DMA is a plain
(cast-free) HWDGE transfer and on-device HBM traffic is halved: 9.2 MiB
per core vs 18.4 fp32.  The SDMA engines are bus-limited at ~27.5 GB/s of
read+write bytes each, so halving bytes on both sides is the main lever.

Per-core structure:
  - x tiles: one [128, 2048] bf16 tile per n holding [x ; x^2] stacked
    along the contraction dim (x^2 squared on ScalarE into the other
    half; layout flipped for odd n so the two DMA halves alternate SBUF
    port halves).
  - act tiles: [128, 2048] bf16 tiles covering 128 consecutive (n,c)
    rows of the [16*80, 2048] act slice - full-width DMAs and full-width
    DVE ops.  The 80-vs-128 misalignment is handled per tile by 1-3
    "n-runs": each run gets its own K=128 matmul with a column slice of
    the [-2c ; 1] weights, writing its partition range of the PSUM tile.
  - One fused DVE scalar_tensor_tensor per [128, 1024] PSUM tile computes
    (dist' + c2) * act and row-sums into a column of a [128, 20]
    accumulator (c2 mapped per (tile, partition) on the host).
  - Tail: reduce the accumulator, ones^T @ racc -> [1,1] on PE.
    Host sums the 8 per-core partials and applies the clip.
  - All tile pools are sized so no buffer is ever recycled: every DMA
    issues with no waits and the HWDGE ring streams back-to-back.
"""

import os
import sys

import numpy as np

for _p in ("/opt/trn_rl_repo",):
    if _p not in sys.path and os.path.isdir(_p):
        sys.path.insert(0, _p)

N, D, C, W = 16, 64, 80, 16384
NCORES = 8
WC = W // NCORES  # 2048 columns per core
NT = (N * C) // 128  # 10 act tiles of 128 rows
SUB = 1024  # free-dim per PSUM tile / fused DVE op
MMN = 512  # matmul free dim (one PSUM bank)
NACC = 2 * NT  # 20 accumulator columns

_CACHE = {}


def _runs():
    """Per act tile t: list of (p0, n, c0, L) n-runs covering its 128 rows."""
    out = []
    for t in range(NT):
        r0 = t * 128
        runs = []
        r = r0
        while r < r0 + 128:
            n, c0 = divmod(r, C)
            L = min(C - c0, r0 + 128 - r)
            runs.append((r - r0, n, c0, L))
            r += L
        out.append(runs)
    return out


def _build_bass():
    import concourse.bacc as bacc
    import concourse.tile as tile
    from concourse import mybir

    fp32 = mybir.dt.float32
    bf16 = mybir.dt.bfloat16
    Alu = mybir.AluOpType

    nc = bacc.Bacc("TRN2", target_bir_lowering=False)

    from contextlib import ExitStack

    RUNS = _runs()
    NRUNS = sum(len(r) for r in RUNS)  # 24

    xs = nc.dram_tensor("xs", [N * D, WC], bf16, kind="ExternalInput")
    acts = nc.dram_tensor("acts", [N * C, WC], bf16, kind="ExternalInput")
    # wR: one zero-padded [128, 128] weight block per n-run (PSUM base
    # partition must be quadrant-aligned, so every matmul spans all 128
    # output partitions and runs accumulate into the same bank).
    wR = nc.dram_tensor("wR", [128, NRUNS * 128], bf16, kind="ExternalInput")
    wtc = nc.dram_tensor("wtc", [128, NT + 1], fp32, kind="ExternalInput")
    out = nc.dram_tensor("out", [1, 1], fp32, kind="ExternalOutput")

    with tile.TileContext(nc) as tc, ExitStack() as ctx:
        consts = ctx.enter_context(tc.tile_pool(name="consts", bufs=1))
        xpool = ctx.enter_context(tc.tile_pool(name="xpool", bufs=N))
        apool = ctx.enter_context(tc.tile_pool(name="apool", bufs=NT))
        spool = ctx.enter_context(tc.tile_pool(name="spool", bufs=2))
        rpool = ctx.enter_context(tc.tile_pool(name="rpool", bufs=1))
        opool = ctx.enter_context(tc.tile_pool(name="opool", bufs=1))
        pdist = ctx.enter_context(tc.tile_pool(name="pdist", bufs=3, space="PSUM"))
        psmall = ctx.enter_context(tc.tile_pool(name="psmall", bufs=1, space="PSUM"))

        wR_t = consts.tile([128, NRUNS * 128], bf16)
        nc.sync.dma_start(out=wR_t[:], in_=wR[:, :])
        wtc_t = consts.tile([128, NT + 1], fp32)
        nc.sync.dma_start(out=wtc_t[:], in_=wtc[:, :])

        racc_all = rpool.tile([128, NACC], fp32)

        xx = {}

        def load_x(n):
            t_ = xpool.tile([128, WC], bf16, tag="xx")
            flip = n % 2 == 1
            # First two n: DMA+square in 512-col slices so the first
            # matmuls can start ~4x sooner (pipeline fill).
            nslc = 4 if n < 2 else 1
            for j in range(nslc):
                s0, s1 = j * (WC // nslc), (j + 1) * (WC // nslc)
                if not flip:
                    nc.sync.dma_start(
                        out=t_[0:D, s0:s1], in_=xs[n * D : (n + 1) * D, s0:s1]
                    )
                    nc.scalar.square(out=t_[D : 2 * D, s0:s1], in_=t_[0:D, s0:s1])
                else:
                    nc.sync.dma_start(
                        out=t_[D : 2 * D, s0:s1], in_=xs[n * D : (n + 1) * D, s0:s1]
                    )
                    nc.scalar.square(out=t_[0:D, s0:s1], in_=t_[D : 2 * D, s0:s1])
            xx[n] = t_

        at = {}

        def load_act(t):
            a_ = apool.tile([128, WC], bf16, tag="at")
            nslc = 2 if t == 0 else 1
            for j in range(nslc):
                s0, s1 = j * (WC // nslc), (j + 1) * (WC // nslc)
                nc.sync.dma_start(
                    out=a_[:, s0:s1], in_=acts[t * 128 : (t + 1) * 128, s0:s1]
                )
            at[t] = a_

        iacc = 0
        ri0 = 0  # index of tile t's first run within the flat run list
        for t in range(NT):
            for (_p0, n, _c0, _L) in RUNS[t]:
                if n not in xx:
                    load_x(n)
            load_act(t)
            nrun = len(RUNS[t])
            for si in range(WC // SUB):
                pd = pdist.tile([128, SUB], fp32, tag="pd")
                for mi in range(SUB // MMN):
                    col = si * SUB + mi * MMN
                    for k, (_p0, n, _c0, _L) in enumerate(RUNS[t]):
                        ri = ri0 + k
                        nc.tensor.matmul(
                            pd[:, mi * MMN : (mi + 1) * MMN],
                            wR_t[:, ri * 128 : (ri + 1) * 128],
                            xx[n][:, col : col + MMN],
                            start=(k == 0),
                            stop=(k == nrun - 1),
                        )
                scr = spool.tile([128, SUB], fp32, tag="scr")
                nc.vector.scalar_tensor_tensor(
                    out=scr[:],
                    in0=pd[:],
                    scalar=wtc_t[:, t : t + 1],
                    in1=at[t][:, si * SUB : (si + 1) * SUB],
                    op0=Alu.add,
                    op1=Alu.mult,
                    accum_out=racc_all[:, iacc : iacc + 1],
                )
                iacc += 1
            ri0 += nrun

        # loss_core = ones^T @ (row-sums of racc_all)
        racc = opool.tile([128, 1], fp32, tag="racc")
        nc.vector.tensor_reduce(
            out=racc[:], in_=racc_all[:], axis=mybir.AxisListType.X, op=Alu.add
        )
        pfin = psmall.tile([1, 1], fp32)
        nc.tensor.matmul(pfin[:], wtc_t[:, NT : NT + 1], racc[:], start=True, stop=True)
        osb = opool.tile([1, 1], fp32, tag="osb")
        nc.vector.tensor_copy(osb[:], pfin[:])
        nc.sync.dma_start(out=out[:, :], in_=osb[:])

    nc.compile()
    return nc


def _get_nc():
    if "nc" not in _CACHE:
        _CACHE["nc"] = _build_bass()
    return _CACHE["nc"]


def prepare_in_maps(x, c, act):
    import ml_dtypes

    bf16 = ml_dtypes.bfloat16
    x = np.ascontiguousarray(np.asarray(x), dtype=np.float32)
    c = np.ascontiguousarray(np.asarray(c), dtype=np.float32)
    act = np.ascontiguousarray(np.asarray(act), dtype=np.float32)
    assert x.shape == (N, D, W) and c.shape == (D, C) and act.shape == (N, C, W)

    xb = x.astype(bf16)
    ab = act.astype(bf16)
    c2 = np.sum(c * c, axis=0, dtype=np.float32)  # [C]
    ones_dc = np.ones((D, C), dtype=np.float32)
    wA = np.concatenate([-2.0 * c, ones_dc], axis=0)  # [128, C], even n
    wB = np.concatenate([ones_dc, -2.0 * c], axis=0)  # [128, C], odd n
    # Zero-padded per-run weight blocks (see wR comment in _build_bass).
    RUNS = _runs()
    blocks = []
    for t in range(NT):
        for (p0, n, c0, L) in RUNS[t]:
            blk = np.zeros((128, 128), dtype=np.float32)
            w_n = wB if n % 2 else wA
            blk[:, p0 : p0 + L] = w_n[:, c0 : c0 + L]
            blocks.append(blk)
    wRh = np.ascontiguousarray(np.concatenate(blocks, axis=1), dtype=bf16)
    # wtc[p, t] = c2[(t*128+p) % C] for act tile t; last col = ones.
    wtc = np.ones((128, NT + 1), dtype=np.float32)
    p = np.arange(128)
    for t in range(NT):
        wtc[:, t] = c2[(t * 128 + p) % C]
    wtc = np.ascontiguousarray(wtc)

    in_maps = []
    for k in range(NCORES):
        sl = slice(k * WC, (k + 1) * WC)
        in_maps.append(
            {
                "xs": np.ascontiguousarray(xb[:, :, sl]).reshape(N * D, WC),
                "acts": np.ascontiguousarray(ab[:, :, sl]).reshape(N * C, WC),
                "wR": wRh,
                "wtc": wtc,
            }
        )
    return in_maps


def kernel(x, c, act):
    from concourse.bass_utils import run_bass_kernel_spmd

    in_maps = prepare_in_maps(x, c, act)
    res = run_bass_kernel_spmd(_get_nc(), in_maps, core_ids=list(range(NCORES)))
    total = np.float32(0.0)
    for r in res.results:
        total = np.float32(total + np.float32(r["out"][0, 0]))
    return np.maximum(np.float32(total), np.float32(1e-6))
